# revision 7
# baseline (speedup 1.0000x reference)
"""AdvancedGraphSAGE (2-layer hetero SAGE + BatchNorm/ReLU) on 8 TRN2 cores.

Strategy (dst-sharded graph parallelism, projection-first layer 1):
  - Nodes sharded 6250/core. Each core owns all edges whose dst is local
    (so local in-degree == global in-degree).
  - Neighbor mean = one-hot segment matmul: gather x[src] rows in 128-edge
    tiles (dma_gather, int16 idx, fp16 256B rows, GB tiles per gather),
    build a weighted one-hot [edge x dst-window] on DVE (iota == dst_off)
    * (1/deg), TensorE contracts over edges.
  - Layer 0 produces feature-major meanT [128 x NPAD]; hT via 3 matmuls.
  - BatchNorm: per-core partial sums -> AllReduce [128,2] -> affine+ReLU
    fused on ScalarE. Layer-0 biases dropped (BN cancels them exactly).
  - Layer 1 is projected BEFORE exchange: p = relu(bn(h)) @ [Wn1s|Wn1a]
    ([dst,32] fp16, per-block matmul straight from hT). AllGather moves
    only [N,32] (3.2MB) into the first 32 columns of a padded [N,128]
    gather table (rows 256B as dma_gather requires; pad cols never read
    by compute, table bitcast to uint16 so pad bytes 0xFF stay finite).
  - Layer 1 aggregation reuses the same edge metadata (same graph):
    gather p rows, per-block psum [128,16] accumulates self-term matmul
    (lhsT=hT block, rhs=W_self1) + one-hot entry matmuls (lhsT=oh,
    rhs=g[...,:16]/[...,16:32]); sim-pass DVE add folds bias, anc-pass
    accumulates; output stored directly from the fp32 accumulator.
All matmul operands are fp16 (PSUM accumulates fp32); everything else fp32.
"""
import sys

if "/opt/trn_rl_repo" not in sys.path:
    sys.path.insert(0, "/opt/trn_rl_repo")

import numpy as np
from contextlib import ExitStack

NCORES = 8
N, E, D, H, C = 50000, 600000, 128, 128, 16
NLOC = N // NCORES
BLK = 128
NB = (NLOC + BLK - 1) // BLK          # 49 blocks; last is 106 wide
NPAD = NB * BLK                        # 6272
HALF = 25000
GB = 8                                 # gather batch, in 128-edge tiles
EPS = 1e-5

_CACHE = {}
_DDS = 16384
_GBUFS = 3
_OHBUFS = 8
_SEGBUFS = 2


def _set_size(n, e, gb=4):
    "Shrink problem size for debugging (call before kernel())."
    global N, E, NLOC, NB, NPAD, HALF, GB
    N, E = n, e
    NLOC = N // NCORES
    NB = (NLOC + BLK - 1) // BLK
    NPAD = NB * BLK
    HALF = N // 2
    GB = gb
    _CACHE.clear()


def _prep_type(src, dst, core):
    """Per-core, per-edge-type stream structure (before cross-core padding).

    Returns dict with per-(block, half) edge arrays in consumption order.
    """
    lo = core * NLOC
    m = (dst >= lo) & (dst < lo + NLOC)
    esrc = src[m].astype(np.int64)
    eoff = (dst[m] - lo).astype(np.int64)
    blk = eoff // BLK
    half = (esrc >= HALF).astype(np.int64)
    order = np.argsort(blk * 2 + half, kind="stable")
    esrc, eoff, blk, half = esrc[order], eoff[order], blk[order], half[order]
    segs = {}
    for b in range(NB):
        for h in range(2):
            sel = (blk == b) & (half == h)
            segs[(b, h)] = (esrc[sel], eoff[sel])
    return segs


def _build_core_arrays(segs, nslots, wglob, lo):
    """Slot-stream layout: per half, blocks packed back-to-back with each
    (block, half) segment padded to the cross-core max slot count (no 128
    rounding); 128-slot tiles may straddle blocks. Pads gather row 0 with
    one-hot weight 0. Returns idx16 per half plus off/w columns per
    (tile, block) entry in shared entry order.
    """
    idx16 = {}
    off_cols, w_cols = [], []
    for h in range(2):
        sidx, soff, sw, sblk = [], [], [], []
        for b in range(NB):
            ns = nslots[(b, h)]
            if ns == 0:
                continue
            esrc, eoff = segs[(b, h)]
            ne = len(esrc)
            bi = np.zeros(ns, np.int16)
            bo = np.full(ns, 300.0, np.float32)
            bw = np.zeros(ns, np.float32)
            bb = np.full(ns, b, np.int64)
            bi[:ne] = (esrc - h * HALF).astype(np.int16)
            bo[:ne] = (eoff - b * BLK).astype(np.float32)
            bw[:ne] = wglob[eoff + lo].astype(np.float32)
            sidx.append(bi); soff.append(bo); sw.append(bw); sblk.append(bb)
        if not sidx:
            sidx, soff, sw, sblk = ([np.zeros(1, np.int16)],
                                    [np.full(1, 300.0, np.float32)],
                                    [np.zeros(1, np.float32)],
                                    [np.zeros(1, np.int64)])
        sidx = np.concatenate(sidx); soff = np.concatenate(soff)
        sw = np.concatenate(sw); sblk = np.concatenate(sblk)
        T = (len(sidx) + 127) // 128
        pad = T * 128 - len(sidx)
        if pad:
            sidx = np.concatenate([sidx, np.zeros(pad, np.int16)])
            soff = np.concatenate([soff, np.full(pad, 300.0, np.float32)])
            sw = np.concatenate([sw, np.zeros(pad, np.float32)])
            sblk = np.concatenate([sblk, np.full(pad, sblk[-1], np.int64)])
        # idx16 wrap: slot i -> [i % 16, i // 16], replicated to 128 rows
        a = np.zeros((16, T * 8), np.int16)
        i = np.arange(T * 128)
        a[i % 16, i // 16] = sidx
        idx16[h] = np.tile(a, (8, 1))
        # entries: per tile, per overlapped block (order must match
        # _entry_plan -- tile-major, block within tile)
        for j in range(T):
            sl = slice(j * 128, (j + 1) * 128)
            blks = np.unique(sblk[sl])
            for b in blks:
                m = sblk[sl] == b
                oc = np.full(128, 300.0, np.float32)
                wc = np.zeros(128, np.float32)
                oc[m] = soff[sl][m]
                wc[m] = sw[sl][m]
                off_cols.append(oc)
                w_cols.append(wc)
    return {"idx0": idx16[0], "idx1": idx16[1],
            "off": np.stack(off_cols, axis=1),
            "w": np.stack(w_cols, axis=1)}


def _entry_plan(nslots):
    """Shared (cross-core) entry plan. Returns per-block matmul entry lists:
    entries[b] = [(h, tile_j, col)] in consumption order, plus per-half tile
    counts. Column indexing matches _build_core_arrays emission order.
    """
    Th = {}
    tile_entries = []          # (h, tile_j, b) in emission order
    for h in range(2):
        sblk = []
        for b in range(NB):
            ns = nslots[(b, h)]
            if ns:
                sblk.append(np.full(ns, b, np.int64))
        sblk = np.concatenate(sblk) if sblk else np.zeros(1, np.int64)
        T = (len(sblk) + 127) // 128
        pad = T * 128 - len(sblk)
        if pad:
            sblk = np.concatenate([sblk, np.full(pad, sblk[-1], np.int64)])
        Th[h] = T
        for j in range(T):
            for b in np.unique(sblk[j * 128:(j + 1) * 128]):
                tile_entries.append((h, j, int(b)))
    entries = {b: [] for b in range(NB)}
    for col, (h, j, b) in enumerate(tile_entries):
        entries[b].append((h, j, col))
    return Th, entries


def _prepare(inputs):
    """Host preprocessing: sharding, sorting, padding, weight combining."""
    x = np.asarray(inputs["x"], np.float32)
    edges = {}
    for t, (ks, kd) in (("s", ("sim_src", "sim_dst")),
                        ("a", ("anc_src", "anc_dst"))):
        edges[t] = (np.asarray(inputs[ks]).astype(np.int64),
                    np.asarray(inputs[kd]).astype(np.int64))

    wglob = {}
    for t in ("s", "a"):
        deg = np.bincount(edges[t][1], minlength=N).astype(np.float32)
        wglob[t] = 1.0 / np.maximum(deg, 1.0)

    per_core_segs = {t: [_prep_type(*edges[t], c) for c in range(NCORES)]
                     for t in ("s", "a")}
    nslots = {}
    for t in ("s", "a"):
        ns = {}
        for b in range(NB):
            for h in range(2):
                ns[(b, h)] = max(len(per_core_segs[t][c][(b, h)][0])
                                 for c in range(NCORES))
            if ns[(b, 0)] + ns[(b, 1)] == 0:
                ns[(b, 0)] = 1
        nslots[t] = ns

    struct = {t: {"nslots": nslots[t]} for t in ("s", "a")}
    core_arrays = {t: [] for t in ("s", "a")}
    for t in ("s", "a"):
        for c in range(NCORES):
            arr = _build_core_arrays(per_core_segs[t][c], nslots[t],
                                     wglob[t], c * NLOC)
            core_arrays[t].append(arr)
        Th, entries = _entry_plan(nslots[t])
        struct[t]["Tlo"] = Th[0]
        struct[t]["Thi"] = Th[1]
        struct[t]["entries"] = entries
        struct[t]["T"] = core_arrays[t][0]["off"].shape[1]

    f16 = np.float16
    wself0 = (0.5 * (np.asarray(inputs["W_self_sim_0"], np.float32)
                     + np.asarray(inputs["W_self_anc_0"], np.float32))).astype(f16)
    wn0s = (0.5 * np.asarray(inputs["W_neigh_sim_0"], np.float32)).astype(f16)
    wn0a = (0.5 * np.asarray(inputs["W_neigh_anc_0"], np.float32)).astype(f16)
    wself1 = (0.5 * (np.asarray(inputs["W_self_sim_1"], np.float32)
                     + np.asarray(inputs["W_self_anc_1"], np.float32))).astype(f16)
    wn1sa = np.concatenate(
        [0.5 * np.asarray(inputs["W_neigh_sim_1"], np.float32),
         0.5 * np.asarray(inputs["W_neigh_anc_1"], np.float32)],
        axis=1).astype(f16)                                   # [128, 32]
    bias1 = np.broadcast_to(
        0.5 * (np.asarray(inputs["b_sim_1"], np.float32)
               + np.asarray(inputs["b_anc_1"], np.float32)), (128, C)
    ).astype(np.float32).copy()
    gamma = np.asarray(inputs["bn_gamma_0"], np.float32).reshape(128, 1).copy()
    beta = np.asarray(inputs["bn_beta_0"], np.float32).reshape(128, 1).copy()

    xf16 = x.astype(f16)
    xlo, xhi = xf16[:HALF].copy(), xf16[HALF:].copy()

    in_maps = []
    for c in range(NCORES):
        xlT = np.zeros((128, NPAD), f16)
        xlT[:, :NLOC] = x[c * NLOC:(c + 1) * NLOC].T.astype(f16)
        im = {
            "xlo": xlo, "xhi": xhi, "xlT": xlT,
            "wself0": wself0, "wn0s": wn0s, "wn0a": wn0a,
            "wself1": wself1, "wn1sa": wn1sa,
            "bias1": bias1, "gamma": gamma, "beta": beta,
        }
        for t in ("s", "a"):
            arr = core_arrays[t][c]
            im[f"idx_{t}_lo"] = arr["idx0"]
            im[f"idx_{t}_hi"] = arr["idx1"]
            im[f"off_{t}"] = arr["off"]
            im[f"w_{t}"] = arr["w"]
        in_maps.append(im)
    return struct, in_maps


def _build(struct):
    import concourse.bacc as bacc
    import concourse.mybir as mybir
    import concourse.tile as tile
    from concourse import bass

    f16, f32 = mybir.dt.float16, mybir.dt.float32
    u16 = mybir.dt.uint16
    nc = bacc.Bacc(None, num_devices=NCORES, dynamic_dma_scratch_size=_DDS)

    din = {}
    def inp(name, shape, dtype):
        din[name] = nc.dram_tensor(name, shape, dtype, kind="ExternalInput")
        return din[name]

    inp("xlo", [HALF, 128], f16)
    inp("xhi", [HALF, 128], f16)
    inp("xlT", [128, NPAD], f16)
    inp("wself0", [128, 128], f16)
    inp("wn0s", [128, 128], f16)
    inp("wn0a", [128, 128], f16)
    inp("wself1", [128, C], f16)
    inp("wn1sa", [128, 2 * C], f16)
    inp("bias1", [128, C], f32)
    inp("gamma", [128, 1], f32)
    inp("beta", [128, 1], f32)
    for t in ("s", "a"):
        st = struct[t]
        inp(f"idx_{t}_lo", [128, max(st["Tlo"], 1) * 8], mybir.dt.int16)
        inp(f"idx_{t}_hi", [128, max(st["Thi"], 1) * 8], mybir.dt.int16)
        inp(f"off_{t}", [128, st["T"]], f32)
        inp(f"w_{t}", [128, st["T"]], f32)
    out_d = nc.dram_tensor("out", [NLOC, C], f32, kind="ExternalOutput")

    with tile.TileContext(nc) as tc, ExitStack() as ctx:
        per = ctx.enter_context(tc.tile_pool(name="per", bufs=1))
        gp = ctx.enter_context(tc.tile_pool(name="gp", bufs=_GBUFS))
        ohp = ctx.enter_context(tc.tile_pool(name="ohp", bufs=_OHBUFS))
        sm = ctx.enter_context(tc.tile_pool(name="sm", bufs=2))
        ps = ctx.enter_context(tc.tile_pool(name="ps", bufs=2, space="PSUM"))
        p1 = ctx.enter_context(tc.tile_pool(name="p1", bufs=2, space="PSUM"))
        dr = ctx.enter_context(tc.tile_pool(name="dr", bufs=1, space="DRAM"))

        def load(name):
            d = din[name]
            t = per.tile(list(d.shape), d.dtype, tag=name)
            nc.sync.dma_start(out=t[:], in_=d[:, :])
            return t

        sb = {k: load(k) for k in
              ["xlT", "wself0", "wn0s", "wn0a", "wself1", "wn1sa",
               "bias1", "gamma", "beta",
               "idx_s_lo", "idx_s_hi", "idx_a_lo", "idx_a_hi",
               "off_s", "w_s", "off_a", "w_a"]}
        iota = per.tile([128, 128], f16, tag="iota")
        ioti = per.tile([128, 128], mybir.dt.int16, tag="ioti")
        nc.gpsimd.iota(ioti[:], pattern=[[1, 128]], base=0,
                       channel_multiplier=0)
        nc.vector.tensor_copy(out=iota[:], in_=ioti[:])

        nireg_cache = {}
        def nireg(v):
            if v not in nireg_cache:
                nireg_cache[v] = nc.gpsimd.to_reg(v)
            return nireg_cache[v]

        hT = per.tile([128, NPAD], f16, tag="hT")
        mean_s = per.tile([128, NPAD], f16, tag="mean_s")
        mean_a = per.tile([128, NPAD], f16, tag="mean_a")
        sums = per.tile([128, NB], f32, tag="sums")
        sumsq = per.tile([128, NB], f32, tag="sumsq")
        stag = per.tile([128, NB * C], f32, tag="stag")

        ploc = dr.tile([NLOC, 2 * C], f16)
        pfullc = dr.tile([N, 2 * C], f16)
        pfull = dr.tile([N, 128], f16)
        bnin = dr.tile([128, 2], f32)
        bnout = dr.tile([128, 2], f32)

        def gather_stream(t, tlo_ap, thi_ap):
            """Generator-side gather machinery shared by both layers."""
            st = struct[t]
            idx = {0: sb[f"idx_{t}_lo"], 1: sb[f"idx_{t}_hi"]}
            tot = {0: st["Tlo"], 1: st["Thi"]}
            tabs = {0: tlo_ap, 1: thi_ap}
            gbuf = {0: [], 1: []}
            emitted = {0: 0, 1: 0}

            def ensure(h, batch):
                while emitted[h] <= batch:
                    k = emitted[h]
                    nb_t = min(GB, tot[h] - k * GB)
                    g = gp.tile([128, GB, 128], u16, tag="g")
                    nc.gpsimd.dma_gather(
                        out_ap=g[:, :nb_t, :],
                        in_ap=tabs[h],
                        idxs_ap=idx[h][:, k * GB * 8:(k * GB + nb_t) * 8],
                        num_idxs=nb_t * 128,
                        num_idxs_reg=nireg(nb_t * 128),
                        elem_size=128,
                    )
                    gbuf[h].append(g)
                    emitted[h] += 1

            def get(h, j):
                batch, slot = j // GB, j % GB
                ensure(h, batch)
                return gbuf[h][batch], slot

            return get

        def onehot(t, col):
            oh = ohp.tile([128, BLK], f16, tag="oh")
            nc.vector.tensor_scalar(
                out=oh[:], in0=iota[:],
                scalar1=sb[f"off_{t}"][:, col:col + 1],
                scalar2=sb[f"w_{t}"][:, col:col + 1],
                op0=mybir.AluOpType.is_equal,
                op1=mybir.AluOpType.mult)
            return oh

        # ---------------- layer 0: neighbor means ----------------
        for t, mean in (("s", mean_s), ("a", mean_a)):
            get = gather_stream(t, din["xlo"][:, :].bitcast(u16),
                                din["xhi"][:, :].bitcast(u16))
            for b in range(NB):
                ents = struct[t]["entries"][b]
                psum = ps.tile([128, BLK], f32, tag="seg", bufs=_SEGBUFS)
                for k, (h, j, col) in enumerate(ents):
                    g, slot = get(h, j)
                    oh = onehot(t, col)
                    nc.tensor.matmul(out=psum[:],
                                     lhsT=g[:, slot, :].bitcast(f16),
                                     rhs=oh[:], start=(k == 0),
                                     stop=(k == len(ents) - 1))
                nc.scalar.activation(mean[:, b * BLK:(b + 1) * BLK],
                                     psum[:],
                                     mybir.ActivationFunctionType.Copy)

        # ---------------- layer 0: hT blocks + BN stats ----------------
        for b in range(NB):
            cols = slice(b * BLK, (b + 1) * BLK)
            po = ps.tile([128, BLK], f32, tag="oT")
            nc.tensor.matmul(out=po[:], lhsT=sb["wself0"][:], rhs=sb["xlT"][:, cols],
                             start=True, stop=False)
            nc.tensor.matmul(out=po[:], lhsT=sb["wn0s"][:], rhs=mean_s[:, cols],
                             start=False, stop=False)
            nc.tensor.matmul(out=po[:], lhsT=sb["wn0a"][:], rhs=mean_a[:, cols],
                             start=False, stop=True)
            nc.scalar.activation(hT[:, cols], po[:],
                                 mybir.ActivationFunctionType.Copy)
            nc.vector.tensor_reduce(out=sums[:, b:b + 1], in_=hT[:, cols],
                                    axis=mybir.AxisListType.X,
                                    op=mybir.AluOpType.add)
            sq = sm.tile([128, BLK], f32, tag="sq")
            nc.vector.tensor_tensor(out=sq[:], in0=hT[:, cols],
                                    in1=hT[:, cols],
                                    op=mybir.AluOpType.mult)
            nc.vector.tensor_reduce(out=sumsq[:, b:b + 1], in_=sq[:],
                                    axis=mybir.AxisListType.X,
                                    op=mybir.AluOpType.add)

        # ---------------- batchnorm ----------------
        bnv = per.tile([128, 2], f32, tag="bnv")
        nc.vector.tensor_reduce(out=bnv[:, 0:1], in_=sums[:],
                                axis=mybir.AxisListType.X,
                                op=mybir.AluOpType.add)
        nc.vector.tensor_reduce(out=bnv[:, 1:2], in_=sumsq[:],
                                axis=mybir.AxisListType.X,
                                op=mybir.AluOpType.add)
        nc.sync.dma_start(out=bnin[:], in_=bnv[:])
        nc.gpsimd.collective_compute(
            "AllReduce", mybir.AluOpType.add,
            replica_groups=[list(range(NCORES))],
            ins=[bnin[:].opt()], outs=[bnout[:].opt()])
        bng = per.tile([128, 2], f32, tag="bng")
        nc.sync.dma_start(out=bng[:], in_=bnout[:])
        mu = per.tile([128, 1], f32, tag="mu")
        ex2 = per.tile([128, 1], f32, tag="ex2")
        var = per.tile([128, 1], f32, tag="var")
        sd = per.tile([128, 1], f32, tag="sd")
        rs = per.tile([128, 1], f32, tag="rs")
        av = per.tile([128, 1], f32, tag="av")
        bv = per.tile([128, 1], f32, tag="bv")
        tmp = per.tile([128, 1], f32, tag="tmp")
        nc.vector.tensor_scalar_mul(mu[:], bng[:, 0:1], 1.0 / N)
        nc.vector.tensor_scalar_mul(ex2[:], bng[:, 1:2], 1.0 / N)
        nc.vector.tensor_tensor(out=tmp[:], in0=mu[:], in1=mu[:],
                                op=mybir.AluOpType.mult)
        nc.vector.tensor_tensor(out=var[:], in0=ex2[:], in1=tmp[:],
                                op=mybir.AluOpType.subtract)
        nc.vector.tensor_scalar_add(var[:], var[:], EPS)
        nc.scalar.activation(sd[:], var[:], mybir.ActivationFunctionType.Sqrt)
        nc.vector.reciprocal(rs[:], sd[:])
        nc.vector.tensor_tensor(out=av[:], in0=sb["gamma"][:], in1=rs[:],
                                op=mybir.AluOpType.mult)
        nc.vector.tensor_tensor(out=tmp[:], in0=av[:], in1=mu[:],
                                op=mybir.AluOpType.mult)
        nc.vector.tensor_tensor(out=bv[:], in0=sb["beta"][:], in1=tmp[:],
                                op=mybir.AluOpType.subtract)
        nc.scalar.activation(hT[:], hT[:], mybir.ActivationFunctionType.Relu,
                             scale=av[:], bias=bv[:])

        # ---------------- projection p = relu(bn(h)) @ [Wn1s|Wn1a] ------
        for b in range(NB):
            cols = slice(b * BLK, (b + 1) * BLK)
            bw = min(BLK, NLOC - b * BLK)
            pp = ps.tile([128, 2 * C], f32, tag="pp")
            nc.tensor.matmul(out=pp[:], lhsT=hT[:, cols], rhs=sb["wn1sa"][:],
                             start=True, stop=True)
            pn = sm.tile([128, 2 * C], f16, tag="pn")
            nc.scalar.activation(pn[:], pp[:],
                                 mybir.ActivationFunctionType.Copy)
            nc.sync.dma_start(out=ploc[b * BLK:b * BLK + bw, :],
                              in_=pn[:bw, :])

        nc.gpsimd.collective_compute(
            "AllGather", mybir.AluOpType.bypass,
            replica_groups=[list(range(NCORES))],
            ins=[ploc[:].opt()], outs=[pfullc[:].opt()])
        nc.sync.dma_start(out=pfull[:, 0:2 * C], in_=pfullc[:, :])

        # ---------------- layer 1 ----------------
        # sim pass: psum = self + sim entries; stag = psum + bias
        # anc pass: psum = anc entries;        stag = stag + psum
        for t in ("s", "a"):
            get = gather_stream(
                t,
                pfull[0:HALF, :].bitcast(u16),
                pfull[HALF:N, :].bitcast(u16))
            fsl = slice(0, C) if t == "s" else slice(C, 2 * C)
            for b in range(NB):
                cols = slice(b * BLK, (b + 1) * BLK)
                osl = slice(b * C, (b + 1) * C)
                ents = struct[t]["entries"][b]
                pf = p1.tile([128, C], f32, tag="pf")
                if t == "s":
                    nc.tensor.matmul(out=pf[:], lhsT=hT[:, cols],
                                     rhs=sb["wself1"][:],
                                     start=True, stop=(len(ents) == 0))
                for k, (h, j, col) in enumerate(ents):
                    g, slot = get(h, j)
                    oh = onehot(t, col)
                    nc.tensor.matmul(out=pf[:], lhsT=oh[:],
                                     rhs=g[:, slot, fsl].bitcast(f16),
                                     start=(t == "a" and k == 0),
                                     stop=(k == len(ents) - 1))
                if t == "s":
                    nc.vector.tensor_tensor(out=stag[:, osl], in0=pf[:],
                                            in1=sb["bias1"][:],
                                            op=mybir.AluOpType.add)
                else:
                    nc.vector.tensor_tensor(out=stag[:, osl], in0=stag[:, osl],
                                            in1=pf[:],
                                            op=mybir.AluOpType.add)
        for b in range(NB):
            bw = min(BLK, NLOC - b * BLK)
            nc.sync.dma_start(out=out_d[b * BLK:b * BLK + bw, :],
                              in_=stag[:bw, b * C:(b + 1) * C])
    nc.compile()
    return nc


def kernel(**inputs):
    from concourse.bass_utils import run_bass_kernel_spmd

    struct, in_maps = _prepare(inputs)
    key = (tuple(sorted(struct["s"]["nslots"].items())),
           tuple(sorted(struct["a"]["nslots"].items())))
    if key not in _CACHE:
        _CACHE.clear()
        _CACHE[key] = _build(struct)
    nc = _CACHE[key]
    res = run_bass_kernel_spmd(nc, in_maps, core_ids=list(range(NCORES)))
    return np.concatenate([res.results[c]["out"] for c in range(NCORES)],
                          axis=0)


# revision 8
# speedup vs baseline: 1.5134x; 1.5134x over previous
"""AdvancedGraphSAGE (2-layer hetero SAGE + BatchNorm/ReLU) on 8 TRN2 cores.

Strategy (dst-sharded graph parallelism, projection-first layer 1):
  - Nodes sharded 6250/core. Each core owns all edges whose dst is local
    (so local in-degree == global in-degree).
  - Neighbor mean = one-hot segment matmul: gather x[src] rows in 128-edge
    tiles (dma_gather, int16 idx, fp16 256B rows, GB tiles per gather),
    build a weighted one-hot [edge x dst-window] on DVE (iota == dst_off)
    * (1/deg), TensorE contracts over edges.
  - Layer 0 produces feature-major meanT [128 x NPAD]; hT via 3 matmuls.
  - BatchNorm: per-core partial sums -> AllReduce [128,2] -> affine+ReLU
    fused on ScalarE. Layer-0 biases dropped (BN cancels them exactly).
  - Layer 1 is projected BEFORE exchange: p = relu(bn(h)) @ [Wn1s|Wn1a]
    ([dst,32] fp16, per-block matmul straight from hT). AllGather moves
    only [N,32] (3.2MB) into the first 32 columns of a padded [N,128]
    gather table (rows 256B as dma_gather requires; pad cols never read
    by compute, table bitcast to uint16 so pad bytes 0xFF stay finite).
  - Layer 1 aggregation reuses the same edge metadata (same graph):
    gather p rows, per-block psum [128,16] accumulates self-term matmul
    (lhsT=hT block, rhs=W_self1) + one-hot entry matmuls (lhsT=oh,
    rhs=g[...,:16]/[...,16:32]); sim-pass DVE add folds bias, anc-pass
    accumulates; output stored directly from the fp32 accumulator.
All matmul operands are fp16 (PSUM accumulates fp32); everything else fp32.
"""
import sys

if "/opt/trn_rl_repo" not in sys.path:
    sys.path.insert(0, "/opt/trn_rl_repo")

import numpy as np
from contextlib import ExitStack

NCORES = 8
N, E, D, H, C = 50000, 600000, 128, 128, 16
NLOC = N // NCORES
BLK = 128
NB = (NLOC + BLK - 1) // BLK          # 49 blocks; last is 106 wide
NPAD = NB * BLK                        # 6272
HALF = 25000
GB = 8                                 # gather batch, in 128-edge tiles
EPS = 1e-5

_CACHE = {}
_DDS = 16384
_GBUFS = 4
_OHBUFS = 8
_SEGBUFS = 2


def _set_size(n, e, gb=4):
    "Shrink problem size for debugging (call before kernel())."
    global N, E, NLOC, NB, NPAD, HALF, GB
    N, E = n, e
    NLOC = N // NCORES
    NB = (NLOC + BLK - 1) // BLK
    NPAD = NB * BLK
    HALF = N // 2
    GB = gb
    _CACHE.clear()


def _prep_type(src, dst, core):
    """Per-core, per-edge-type stream structure (before cross-core padding).

    Returns dict with per-(block, half) edge arrays in consumption order.
    """
    lo = core * NLOC
    m = (dst >= lo) & (dst < lo + NLOC)
    esrc = src[m].astype(np.int64)
    eoff = (dst[m] - lo).astype(np.int64)
    blk = eoff // BLK
    half = (esrc >= HALF).astype(np.int64)
    order = np.argsort(blk * 2 + half, kind="stable")
    esrc, eoff, blk, half = esrc[order], eoff[order], blk[order], half[order]
    segs = {}
    for b in range(NB):
        for h in range(2):
            sel = (blk == b) & (half == h)
            segs[(b, h)] = (esrc[sel], eoff[sel])
    return segs


def _build_core_arrays(segs, nslots, wglob, lo):
    """Slot-stream layout: per half, blocks packed back-to-back with each
    (block, half) segment padded to the cross-core max slot count (no 128
    rounding); 128-slot tiles may straddle blocks. Pads gather row 0 with
    one-hot weight 0. Returns idx16 per half plus off/w columns per
    (tile, block) entry in shared entry order.
    """
    idx16 = {}
    off_cols, w_cols = [], []
    for h in range(2):
        sidx, soff, sw, sblk = [], [], [], []
        for b in range(NB):
            ns = nslots[(b, h)]
            if ns == 0:
                continue
            esrc, eoff = segs[(b, h)]
            ne = len(esrc)
            bi = np.zeros(ns, np.int16)
            bo = np.full(ns, 300.0, np.float32)
            bw = np.zeros(ns, np.float32)
            bb = np.full(ns, b, np.int64)
            bi[:ne] = (esrc - h * HALF).astype(np.int16)
            bo[:ne] = (eoff - b * BLK).astype(np.float32)
            bw[:ne] = wglob[eoff + lo].astype(np.float32)
            sidx.append(bi); soff.append(bo); sw.append(bw); sblk.append(bb)
        if not sidx:
            sidx, soff, sw, sblk = ([np.zeros(1, np.int16)],
                                    [np.full(1, 300.0, np.float32)],
                                    [np.zeros(1, np.float32)],
                                    [np.zeros(1, np.int64)])
        sidx = np.concatenate(sidx); soff = np.concatenate(soff)
        sw = np.concatenate(sw); sblk = np.concatenate(sblk)
        T = (len(sidx) + 127) // 128
        pad = T * 128 - len(sidx)
        if pad:
            sidx = np.concatenate([sidx, np.zeros(pad, np.int16)])
            soff = np.concatenate([soff, np.full(pad, 300.0, np.float32)])
            sw = np.concatenate([sw, np.zeros(pad, np.float32)])
            sblk = np.concatenate([sblk, np.full(pad, sblk[-1], np.int64)])
        # idx16 wrap: slot i -> [i % 16, i // 16], replicated to 128 rows
        a = np.zeros((16, T * 8), np.int16)
        i = np.arange(T * 128)
        a[i % 16, i // 16] = sidx
        idx16[h] = np.tile(a, (8, 1))
        # entries: per tile, per overlapped block (order must match
        # _entry_plan -- tile-major, block within tile)
        for j in range(T):
            sl = slice(j * 128, (j + 1) * 128)
            blks = np.unique(sblk[sl])
            for b in blks:
                m = sblk[sl] == b
                oc = np.full(128, 300.0, np.float32)
                wc = np.zeros(128, np.float32)
                oc[m] = soff[sl][m]
                wc[m] = sw[sl][m]
                off_cols.append(oc)
                w_cols.append(wc)
    return {"idx0": idx16[0], "idx1": idx16[1],
            "off": np.stack(off_cols, axis=1),
            "w": np.stack(w_cols, axis=1)}


def _entry_plan(nslots):
    """Shared (cross-core) entry plan. Returns per-block matmul entry lists:
    entries[b] = [(h, tile_j, col)] in consumption order, plus per-half tile
    counts. Column indexing matches _build_core_arrays emission order.
    """
    Th = {}
    tile_entries = []          # (h, tile_j, b) in emission order
    for h in range(2):
        sblk = []
        for b in range(NB):
            ns = nslots[(b, h)]
            if ns:
                sblk.append(np.full(ns, b, np.int64))
        sblk = np.concatenate(sblk) if sblk else np.zeros(1, np.int64)
        T = (len(sblk) + 127) // 128
        pad = T * 128 - len(sblk)
        if pad:
            sblk = np.concatenate([sblk, np.full(pad, sblk[-1], np.int64)])
        Th[h] = T
        for j in range(T):
            for b in np.unique(sblk[j * 128:(j + 1) * 128]):
                tile_entries.append((h, j, int(b)))
    entries = {b: [] for b in range(NB)}
    for col, (h, j, b) in enumerate(tile_entries):
        entries[b].append((h, j, col))
    return Th, entries


def _prepare(inputs):
    """Host preprocessing: sharding, sorting, padding, weight combining."""
    x = np.asarray(inputs["x"], np.float32)
    edges = {}
    for t, (ks, kd) in (("s", ("sim_src", "sim_dst")),
                        ("a", ("anc_src", "anc_dst"))):
        edges[t] = (np.asarray(inputs[ks]).astype(np.int64),
                    np.asarray(inputs[kd]).astype(np.int64))

    wglob = {}
    for t in ("s", "a"):
        deg = np.bincount(edges[t][1], minlength=N).astype(np.float32)
        wglob[t] = 1.0 / np.maximum(deg, 1.0)

    per_core_segs = {t: [_prep_type(*edges[t], c) for c in range(NCORES)]
                     for t in ("s", "a")}
    nslots = {}
    for t in ("s", "a"):
        ns = {}
        for b in range(NB):
            for h in range(2):
                ns[(b, h)] = max(len(per_core_segs[t][c][(b, h)][0])
                                 for c in range(NCORES))
            if ns[(b, 0)] + ns[(b, 1)] == 0:
                ns[(b, 0)] = 1
        nslots[t] = ns

    struct = {t: {"nslots": nslots[t]} for t in ("s", "a")}
    core_arrays = {t: [] for t in ("s", "a")}
    for t in ("s", "a"):
        for c in range(NCORES):
            arr = _build_core_arrays(per_core_segs[t][c], nslots[t],
                                     wglob[t], c * NLOC)
            core_arrays[t].append(arr)
        Th, entries = _entry_plan(nslots[t])
        struct[t]["Tlo"] = Th[0]
        struct[t]["Thi"] = Th[1]
        struct[t]["entries"] = entries
        struct[t]["T"] = core_arrays[t][0]["off"].shape[1]

    f16 = np.float16
    wself0 = (0.5 * (np.asarray(inputs["W_self_sim_0"], np.float32)
                     + np.asarray(inputs["W_self_anc_0"], np.float32))).astype(f16)
    wn0s = (0.5 * np.asarray(inputs["W_neigh_sim_0"], np.float32)).astype(f16)
    wn0a = (0.5 * np.asarray(inputs["W_neigh_anc_0"], np.float32)).astype(f16)
    wself1 = (0.5 * (np.asarray(inputs["W_self_sim_1"], np.float32)
                     + np.asarray(inputs["W_self_anc_1"], np.float32))).astype(f16)
    wn1sa = np.concatenate(
        [0.5 * np.asarray(inputs["W_neigh_sim_1"], np.float32),
         0.5 * np.asarray(inputs["W_neigh_anc_1"], np.float32)],
        axis=1).astype(f16)                                   # [128, 32]
    bias1 = np.broadcast_to(
        0.5 * (np.asarray(inputs["b_sim_1"], np.float32)
               + np.asarray(inputs["b_anc_1"], np.float32)), (128, C)
    ).astype(np.float32).copy()
    gamma = np.asarray(inputs["bn_gamma_0"], np.float32).reshape(128, 1).copy()
    beta = np.asarray(inputs["bn_beta_0"], np.float32).reshape(128, 1).copy()

    xf16 = x.astype(f16)
    xlo, xhi = xf16[:HALF].copy(), xf16[HALF:].copy()

    in_maps = []
    for c in range(NCORES):
        xlT = np.zeros((128, NPAD), f16)
        xlT[:, :NLOC] = x[c * NLOC:(c + 1) * NLOC].T.astype(f16)
        im = {
            "xlo": xlo, "xhi": xhi, "xlT": xlT,
            "wself0": wself0, "wn0s": wn0s, "wn0a": wn0a,
            "wself1": wself1, "wn1sa": wn1sa,
            "bias1": bias1, "gamma": gamma, "beta": beta,
        }
        for t in ("s", "a"):
            arr = core_arrays[t][c]
            im[f"idx_{t}_lo"] = arr["idx0"]
            im[f"idx_{t}_hi"] = arr["idx1"]
            im[f"off_{t}"] = arr["off"]
            im[f"w_{t}"] = arr["w"]
        in_maps.append(im)
    return struct, in_maps


def _build(struct):
    import concourse.bacc as bacc
    import concourse.mybir as mybir
    import concourse.tile as tile
    from concourse import bass

    f16, f32 = mybir.dt.float16, mybir.dt.float32
    u16 = mybir.dt.uint16
    nc = bacc.Bacc(None, num_devices=NCORES, dynamic_dma_scratch_size=_DDS)

    din = {}
    def inp(name, shape, dtype):
        din[name] = nc.dram_tensor(name, shape, dtype, kind="ExternalInput")
        return din[name]

    inp("xlo", [HALF, 128], f16)
    inp("xhi", [HALF, 128], f16)
    inp("xlT", [128, NPAD], f16)
    inp("wself0", [128, 128], f16)
    inp("wn0s", [128, 128], f16)
    inp("wn0a", [128, 128], f16)
    inp("wself1", [128, C], f16)
    inp("wn1sa", [128, 2 * C], f16)
    inp("bias1", [128, C], f32)
    inp("gamma", [128, 1], f32)
    inp("beta", [128, 1], f32)
    for t in ("s", "a"):
        st = struct[t]
        inp(f"idx_{t}_lo", [128, max(st["Tlo"], 1) * 8], mybir.dt.int16)
        inp(f"idx_{t}_hi", [128, max(st["Thi"], 1) * 8], mybir.dt.int16)
        inp(f"off_{t}", [128, st["T"]], f32)
        inp(f"w_{t}", [128, st["T"]], f32)
    out_d = nc.dram_tensor("out", [NLOC, C], f32, kind="ExternalOutput")

    with tile.TileContext(nc) as tc, ExitStack() as ctx:
        per = ctx.enter_context(tc.tile_pool(name="per", bufs=1))
        gp = ctx.enter_context(tc.tile_pool(name="gp", bufs=_GBUFS))
        ohp = ctx.enter_context(tc.tile_pool(name="ohp", bufs=_OHBUFS))
        sm = ctx.enter_context(tc.tile_pool(name="sm", bufs=2))
        ps = ctx.enter_context(tc.tile_pool(name="ps", bufs=2, space="PSUM"))
        p1 = ctx.enter_context(tc.tile_pool(name="p1", bufs=2, space="PSUM"))
        dr = ctx.enter_context(tc.tile_pool(name="dr", bufs=1, space="DRAM"))

        def load(name):
            d = din[name]
            t = per.tile(list(d.shape), d.dtype, tag=name)
            nc.sync.dma_start(out=t[:], in_=d[:, :])
            return t

        sb = {k: load(k) for k in
              ["xlT", "wself0", "wn0s", "wn0a", "wself1", "wn1sa",
               "bias1", "gamma", "beta",
               "idx_s_lo", "idx_s_hi", "idx_a_lo", "idx_a_hi",
               "off_s", "w_s", "off_a", "w_a"]}
        iota = per.tile([128, 128], f16, tag="iota")
        ioti = per.tile([128, 128], mybir.dt.int16, tag="ioti")
        nc.gpsimd.iota(ioti[:], pattern=[[1, 128]], base=0,
                       channel_multiplier=0)
        nc.vector.tensor_copy(out=iota[:], in_=ioti[:])

        nireg_cache = {}
        def nireg(v):
            if v not in nireg_cache:
                nireg_cache[v] = nc.gpsimd.to_reg(v)
            return nireg_cache[v]

        hT = per.tile([128, NPAD], f16, tag="hT")
        mean_s = per.tile([128, NPAD], f16, tag="mean_s")
        mean_a = per.tile([128, NPAD], f16, tag="mean_a")
        sums = per.tile([128, NB], f32, tag="sums")
        sumsq = per.tile([128, NB], f32, tag="sumsq")
        stag = per.tile([128, NB * C], f32, tag="stag")

        ploc = dr.tile([NLOC, 2 * C], f16)
        pfullc = dr.tile([N, 2 * C], f16)
        pfull = dr.tile([N, 128], f16)
        bnin = dr.tile([128, 2], f32)
        bnout = dr.tile([128, 2], f32)

        def gather_stream(t, tlo_ap, thi_ap):
            """Generator-side gather machinery shared by both layers."""
            st = struct[t]
            idx = {0: sb[f"idx_{t}_lo"], 1: sb[f"idx_{t}_hi"]}
            tot = {0: st["Tlo"], 1: st["Thi"]}
            tabs = {0: tlo_ap, 1: thi_ap}
            gbuf = {0: [], 1: []}
            emitted = {0: 0, 1: 0}

            def ensure(h, batch):
                while emitted[h] <= batch:
                    k = emitted[h]
                    nb_t = min(GB, tot[h] - k * GB)
                    g = gp.tile([128, GB, 128], u16, tag=f"g{h}")
                    nc.gpsimd.dma_gather(
                        out_ap=g[:, :nb_t, :],
                        in_ap=tabs[h],
                        idxs_ap=idx[h][:, k * GB * 8:(k * GB + nb_t) * 8],
                        num_idxs=nb_t * 128,
                        num_idxs_reg=nireg(nb_t * 128),
                        elem_size=128,
                    )
                    gbuf[h].append(g)
                    emitted[h] += 1

            def get(h, j):
                batch, slot = j // GB, j % GB
                ensure(h, batch)
                return gbuf[h][batch], slot

            return get

        def onehot(t, col):
            oh = ohp.tile([128, BLK], f16, tag="oh")
            nc.vector.tensor_scalar(
                out=oh[:], in0=iota[:],
                scalar1=sb[f"off_{t}"][:, col:col + 1],
                scalar2=sb[f"w_{t}"][:, col:col + 1],
                op0=mybir.AluOpType.is_equal,
                op1=mybir.AluOpType.mult)
            return oh

        # ---------------- layer 0: neighbor means ----------------
        for t, mean in (("s", mean_s), ("a", mean_a)):
            get = gather_stream(t, din["xlo"][:, :].bitcast(u16),
                                din["xhi"][:, :].bitcast(u16))
            for b in range(NB):
                ents = struct[t]["entries"][b]
                psum = ps.tile([128, BLK], f32, tag="seg", bufs=_SEGBUFS)
                for k, (h, j, col) in enumerate(ents):
                    g, slot = get(h, j)
                    oh = onehot(t, col)
                    nc.tensor.matmul(out=psum[:],
                                     lhsT=g[:, slot, :].bitcast(f16),
                                     rhs=oh[:], start=(k == 0),
                                     stop=(k == len(ents) - 1))
                nc.scalar.activation(mean[:, b * BLK:(b + 1) * BLK],
                                     psum[:],
                                     mybir.ActivationFunctionType.Copy)

        # ---------------- layer 0: hT blocks + BN stats ----------------
        for b in range(NB):
            cols = slice(b * BLK, (b + 1) * BLK)
            po = ps.tile([128, BLK], f32, tag="oT")
            nc.tensor.matmul(out=po[:], lhsT=sb["wself0"][:], rhs=sb["xlT"][:, cols],
                             start=True, stop=False)
            nc.tensor.matmul(out=po[:], lhsT=sb["wn0s"][:], rhs=mean_s[:, cols],
                             start=False, stop=False)
            nc.tensor.matmul(out=po[:], lhsT=sb["wn0a"][:], rhs=mean_a[:, cols],
                             start=False, stop=True)
            nc.scalar.activation(hT[:, cols], po[:],
                                 mybir.ActivationFunctionType.Copy)
            nc.vector.tensor_reduce(out=sums[:, b:b + 1], in_=hT[:, cols],
                                    axis=mybir.AxisListType.X,
                                    op=mybir.AluOpType.add)
            sq = sm.tile([128, BLK], f32, tag="sq")
            nc.vector.tensor_tensor(out=sq[:], in0=hT[:, cols],
                                    in1=hT[:, cols],
                                    op=mybir.AluOpType.mult)
            nc.vector.tensor_reduce(out=sumsq[:, b:b + 1], in_=sq[:],
                                    axis=mybir.AxisListType.X,
                                    op=mybir.AluOpType.add)

        # ---------------- batchnorm ----------------
        bnv = per.tile([128, 2], f32, tag="bnv")
        nc.vector.tensor_reduce(out=bnv[:, 0:1], in_=sums[:],
                                axis=mybir.AxisListType.X,
                                op=mybir.AluOpType.add)
        nc.vector.tensor_reduce(out=bnv[:, 1:2], in_=sumsq[:],
                                axis=mybir.AxisListType.X,
                                op=mybir.AluOpType.add)
        nc.sync.dma_start(out=bnin[:], in_=bnv[:])
        nc.gpsimd.collective_compute(
            "AllReduce", mybir.AluOpType.add,
            replica_groups=[list(range(NCORES))],
            ins=[bnin[:].opt()], outs=[bnout[:].opt()])
        bng = per.tile([128, 2], f32, tag="bng")
        nc.sync.dma_start(out=bng[:], in_=bnout[:])
        mu = per.tile([128, 1], f32, tag="mu")
        ex2 = per.tile([128, 1], f32, tag="ex2")
        var = per.tile([128, 1], f32, tag="var")
        sd = per.tile([128, 1], f32, tag="sd")
        rs = per.tile([128, 1], f32, tag="rs")
        av = per.tile([128, 1], f32, tag="av")
        bv = per.tile([128, 1], f32, tag="bv")
        tmp = per.tile([128, 1], f32, tag="tmp")
        nc.vector.tensor_scalar_mul(mu[:], bng[:, 0:1], 1.0 / N)
        nc.vector.tensor_scalar_mul(ex2[:], bng[:, 1:2], 1.0 / N)
        nc.vector.tensor_tensor(out=tmp[:], in0=mu[:], in1=mu[:],
                                op=mybir.AluOpType.mult)
        nc.vector.tensor_tensor(out=var[:], in0=ex2[:], in1=tmp[:],
                                op=mybir.AluOpType.subtract)
        nc.vector.tensor_scalar_add(var[:], var[:], EPS)
        nc.scalar.activation(sd[:], var[:], mybir.ActivationFunctionType.Sqrt)
        nc.vector.reciprocal(rs[:], sd[:])
        nc.vector.tensor_tensor(out=av[:], in0=sb["gamma"][:], in1=rs[:],
                                op=mybir.AluOpType.mult)
        nc.vector.tensor_tensor(out=tmp[:], in0=av[:], in1=mu[:],
                                op=mybir.AluOpType.mult)
        nc.vector.tensor_tensor(out=bv[:], in0=sb["beta"][:], in1=tmp[:],
                                op=mybir.AluOpType.subtract)
        nc.scalar.activation(hT[:], hT[:], mybir.ActivationFunctionType.Relu,
                             scale=av[:], bias=bv[:])

        # ---------------- projection p = relu(bn(h)) @ [Wn1s|Wn1a] ------
        for b in range(NB):
            cols = slice(b * BLK, (b + 1) * BLK)
            bw = min(BLK, NLOC - b * BLK)
            pp = ps.tile([128, 2 * C], f32, tag="pp")
            nc.tensor.matmul(out=pp[:], lhsT=hT[:, cols], rhs=sb["wn1sa"][:],
                             start=True, stop=True)
            pn = sm.tile([128, 2 * C], f16, tag="pn")
            nc.scalar.activation(pn[:], pp[:],
                                 mybir.ActivationFunctionType.Copy)
            nc.sync.dma_start(out=ploc[b * BLK:b * BLK + bw, :],
                              in_=pn[:bw, :])

        nc.gpsimd.collective_compute(
            "AllGather", mybir.AluOpType.bypass,
            replica_groups=[list(range(NCORES))],
            ins=[ploc[:].opt()], outs=[pfullc[:].opt()])
        nc.sync.dma_start(out=pfull[:, 0:2 * C], in_=pfullc[:, :])

        # ---------------- layer 1 ----------------
        # sim pass: psum = self + sim entries; stag = psum + bias
        # anc pass: psum = anc entries;        stag = stag + psum
        for t in ("s", "a"):
            get = gather_stream(
                t,
                pfull[0:HALF, :].bitcast(u16),
                pfull[HALF:N, :].bitcast(u16))
            fsl = slice(0, C) if t == "s" else slice(C, 2 * C)
            for b in range(NB):
                cols = slice(b * BLK, (b + 1) * BLK)
                osl = slice(b * C, (b + 1) * C)
                ents = struct[t]["entries"][b]
                pf = p1.tile([128, C], f32, tag="pf")
                if t == "s":
                    nc.tensor.matmul(out=pf[:], lhsT=hT[:, cols],
                                     rhs=sb["wself1"][:],
                                     start=True, stop=(len(ents) == 0))
                for k, (h, j, col) in enumerate(ents):
                    g, slot = get(h, j)
                    oh = onehot(t, col)
                    nc.tensor.matmul(out=pf[:], lhsT=oh[:],
                                     rhs=g[:, slot, fsl].bitcast(f16),
                                     start=(t == "a" and k == 0),
                                     stop=(k == len(ents) - 1))
                if t == "s":
                    nc.vector.tensor_tensor(out=stag[:, osl], in0=pf[:],
                                            in1=sb["bias1"][:],
                                            op=mybir.AluOpType.add)
                else:
                    nc.vector.tensor_tensor(out=stag[:, osl], in0=stag[:, osl],
                                            in1=pf[:],
                                            op=mybir.AluOpType.add)
        for b in range(NB):
            bw = min(BLK, NLOC - b * BLK)
            nc.sync.dma_start(out=out_d[b * BLK:b * BLK + bw, :],
                              in_=stag[:bw, b * C:(b + 1) * C])
    nc.compile()
    return nc


def kernel(**inputs):
    from concourse.bass_utils import run_bass_kernel_spmd

    struct, in_maps = _prepare(inputs)
    key = (tuple(sorted(struct["s"]["nslots"].items())),
           tuple(sorted(struct["a"]["nslots"].items())))
    if key not in _CACHE:
        _CACHE.clear()
        _CACHE[key] = _build(struct)
    nc = _CACHE[key]
    res = run_bass_kernel_spmd(nc, in_maps, core_ids=list(range(NCORES)))
    return np.concatenate([res.results[c]["out"] for c in range(NCORES)],
                          axis=0)


# revision 10
# speedup vs baseline: 1.6751x; 1.1068x over previous
"""AdvancedGraphSAGE (2-layer hetero SAGE + BatchNorm/ReLU) on 8 TRN2 cores.

Strategy (dst-sharded graph parallelism, projection-first layer 1):
  - Nodes sharded 6250/core. Each core owns all edges whose dst is local
    (so local in-degree == global in-degree).
  - Neighbor mean = one-hot segment matmul: gather x[src] rows in 128-edge
    tiles (dma_gather, int16 idx, fp16 256B rows, GB tiles per gather),
    build a weighted one-hot [edge x dst-window] on DVE (iota == dst_off)
    * (1/deg), TensorE contracts over edges.
  - Layer 0 produces feature-major meanT [128 x NPAD]; hT via 3 matmuls.
  - BatchNorm: per-core partial sums -> AllReduce [128,2] -> affine+ReLU
    fused on ScalarE. Layer-0 biases dropped (BN cancels them exactly).
  - Layer 1 is projected BEFORE exchange: p = relu(bn(h)) @ [Wn1s|Wn1a]
    ([dst,32] fp16, per-block matmul straight from hT). AllGather moves
    only [N,32] (3.2MB) into the first 32 columns of a padded [N,128]
    gather table (rows 256B as dma_gather requires; pad cols never read
    by compute, table bitcast to uint16 so pad bytes 0xFF stay finite).
  - Layer 1 aggregation reuses the same edge metadata (same graph):
    gather p rows, per-block psum [128,16] accumulates self-term matmul
    (lhsT=hT block, rhs=W_self1) + one-hot entry matmuls (lhsT=oh,
    rhs=g[...,:16]/[...,16:32]); sim-pass DVE add folds bias, anc-pass
    accumulates; output stored directly from the fp32 accumulator.
All matmul operands are fp16 (PSUM accumulates fp32); everything else fp32.
"""
import sys

if "/opt/trn_rl_repo" not in sys.path:
    sys.path.insert(0, "/opt/trn_rl_repo")

import numpy as np
from contextlib import ExitStack

NCORES = 8
N, E, D, H, C = 50000, 600000, 128, 128, 16
NLOC = N // NCORES
BLK = 128
NB = (NLOC + BLK - 1) // BLK          # 49 blocks; last is 106 wide
NPAD = NB * BLK                        # 6272
HALF = 25000
GB = 8                                 # gather batch, in 128-edge tiles
EPS = 1e-5

_CACHE = {}
_DDS = 16384
_GBUFS = 4
_OHBUFS = 8
_SEGBUFS = 2


def _set_size(n, e, gb=4):
    "Shrink problem size for debugging (call before kernel())."
    global N, E, NLOC, NB, NPAD, HALF, GB
    N, E = n, e
    NLOC = N // NCORES
    NB = (NLOC + BLK - 1) // BLK
    NPAD = NB * BLK
    HALF = N // 2
    GB = gb
    _CACHE.clear()


def _prep_type(src, dst, core):
    """Per-core, per-edge-type stream structure (before cross-core padding).

    Returns dict with per-(block, half) edge arrays in consumption order.
    """
    lo = core * NLOC
    m = (dst >= lo) & (dst < lo + NLOC)
    esrc = src[m].astype(np.int64)
    eoff = (dst[m] - lo).astype(np.int64)
    blk = eoff // BLK
    half = (esrc >= HALF).astype(np.int64)
    order = np.argsort(blk * 2 + half, kind="stable")
    esrc, eoff, blk, half = esrc[order], eoff[order], blk[order], half[order]
    segs = {}
    for b in range(NB):
        for h in range(2):
            sel = (blk == b) & (half == h)
            segs[(b, h)] = (esrc[sel], eoff[sel])
    return segs


def _build_core_arrays(segs, nslots, wglob, lo):
    """Slot-stream layout: per half, blocks packed back-to-back with each
    (block, half) segment padded to the cross-core max slot count (no 128
    rounding); 128-slot tiles may straddle blocks. Pads gather row 0 with
    one-hot weight 0. Returns idx16 per half plus off/w columns per
    (tile, block) entry in shared entry order.
    """
    idx16 = {}
    off_cols, w_cols = [], []
    for h in range(2):
        sidx, soff, sw, sblk = [], [], [], []
        for b in range(NB):
            ns = nslots[(b, h)]
            if ns == 0:
                continue
            esrc, eoff = segs[(b, h)]
            ne = len(esrc)
            bi = np.zeros(ns, np.int16)
            bo = np.full(ns, 300.0, np.float32)
            bw = np.zeros(ns, np.float32)
            bb = np.full(ns, b, np.int64)
            bi[:ne] = (esrc - h * HALF).astype(np.int16)
            bo[:ne] = (eoff - b * BLK).astype(np.float32)
            bw[:ne] = wglob[eoff + lo].astype(np.float32)
            sidx.append(bi); soff.append(bo); sw.append(bw); sblk.append(bb)
        if not sidx:
            sidx, soff, sw, sblk = ([np.zeros(1, np.int16)],
                                    [np.full(1, 300.0, np.float32)],
                                    [np.zeros(1, np.float32)],
                                    [np.zeros(1, np.int64)])
        sidx = np.concatenate(sidx); soff = np.concatenate(soff)
        sw = np.concatenate(sw); sblk = np.concatenate(sblk)
        T = (len(sidx) + 127) // 128
        pad = T * 128 - len(sidx)
        if pad:
            sidx = np.concatenate([sidx, np.zeros(pad, np.int16)])
            soff = np.concatenate([soff, np.full(pad, 300.0, np.float32)])
            sw = np.concatenate([sw, np.zeros(pad, np.float32)])
            sblk = np.concatenate([sblk, np.full(pad, sblk[-1], np.int64)])
        # idx16 wrap: slot i -> [i % 16, i // 16], replicated to 128 rows
        a = np.zeros((16, T * 8), np.int16)
        i = np.arange(T * 128)
        a[i % 16, i // 16] = sidx
        idx16[h] = np.tile(a, (8, 1))
        # entries: per tile, per overlapped block (order must match
        # _entry_plan -- tile-major, block within tile)
        for j in range(T):
            sl = slice(j * 128, (j + 1) * 128)
            blks = np.unique(sblk[sl])
            for b in blks:
                m = sblk[sl] == b
                oc = np.full(128, 300.0, np.float32)
                wc = np.zeros(128, np.float32)
                oc[m] = soff[sl][m]
                wc[m] = sw[sl][m]
                off_cols.append(oc)
                w_cols.append(wc)
    return {"idx0": idx16[0], "idx1": idx16[1],
            "off": np.stack(off_cols, axis=1),
            "w": np.stack(w_cols, axis=1)}


def _entry_plan(nslots):
    """Shared (cross-core) entry plan. Returns per-block matmul entry lists:
    entries[b] = [(h, tile_j, col)] in consumption order, plus per-half tile
    counts. Column indexing matches _build_core_arrays emission order.
    """
    Th = {}
    tile_entries = []          # (h, tile_j, b) in emission order
    for h in range(2):
        sblk = []
        for b in range(NB):
            ns = nslots[(b, h)]
            if ns:
                sblk.append(np.full(ns, b, np.int64))
        sblk = np.concatenate(sblk) if sblk else np.zeros(1, np.int64)
        T = (len(sblk) + 127) // 128
        pad = T * 128 - len(sblk)
        if pad:
            sblk = np.concatenate([sblk, np.full(pad, sblk[-1], np.int64)])
        Th[h] = T
        for j in range(T):
            for b in np.unique(sblk[j * 128:(j + 1) * 128]):
                tile_entries.append((h, j, int(b)))
    entries = {b: [] for b in range(NB)}
    for col, (h, j, b) in enumerate(tile_entries):
        entries[b].append((h, j, col))
    return Th, entries


def _prepare(inputs):
    """Host preprocessing: sharding, sorting, padding, weight combining."""
    x = np.asarray(inputs["x"], np.float32)
    edges = {}
    for t, (ks, kd) in (("s", ("sim_src", "sim_dst")),
                        ("a", ("anc_src", "anc_dst"))):
        edges[t] = (np.asarray(inputs[ks]).astype(np.int64),
                    np.asarray(inputs[kd]).astype(np.int64))

    wglob = {}
    for t in ("s", "a"):
        deg = np.bincount(edges[t][1], minlength=N).astype(np.float32)
        wglob[t] = 1.0 / np.maximum(deg, 1.0)

    per_core_segs = {t: [_prep_type(*edges[t], c) for c in range(NCORES)]
                     for t in ("s", "a")}
    nslots = {}
    for t in ("s", "a"):
        ns = {}
        for b in range(NB):
            for h in range(2):
                ns[(b, h)] = max(len(per_core_segs[t][c][(b, h)][0])
                                 for c in range(NCORES))
            if ns[(b, 0)] + ns[(b, 1)] == 0:
                ns[(b, 0)] = 1
        nslots[t] = ns

    struct = {t: {"nslots": nslots[t]} for t in ("s", "a")}
    core_arrays = {t: [] for t in ("s", "a")}
    for t in ("s", "a"):
        for c in range(NCORES):
            arr = _build_core_arrays(per_core_segs[t][c], nslots[t],
                                     wglob[t], c * NLOC)
            core_arrays[t].append(arr)
        Th, entries = _entry_plan(nslots[t])
        struct[t]["Tlo"] = Th[0]
        struct[t]["Thi"] = Th[1]
        struct[t]["entries"] = entries
        struct[t]["T"] = core_arrays[t][0]["off"].shape[1]

    f16 = np.float16
    wself0 = (0.5 * (np.asarray(inputs["W_self_sim_0"], np.float32)
                     + np.asarray(inputs["W_self_anc_0"], np.float32))).astype(f16)
    wn0s = (0.5 * np.asarray(inputs["W_neigh_sim_0"], np.float32)).astype(f16)
    wn0a = (0.5 * np.asarray(inputs["W_neigh_anc_0"], np.float32)).astype(f16)
    wself1 = (0.5 * (np.asarray(inputs["W_self_sim_1"], np.float32)
                     + np.asarray(inputs["W_self_anc_1"], np.float32))).astype(f16)
    wn1sa = np.concatenate(
        [0.5 * np.asarray(inputs["W_neigh_sim_1"], np.float32),
         0.5 * np.asarray(inputs["W_neigh_anc_1"], np.float32)],
        axis=1).astype(f16)                                   # [128, 32]
    bias1 = np.broadcast_to(
        0.5 * (np.asarray(inputs["b_sim_1"], np.float32)
               + np.asarray(inputs["b_anc_1"], np.float32)), (128, C)
    ).astype(np.float32).copy()
    gamma = np.asarray(inputs["bn_gamma_0"], np.float32).reshape(128, 1).copy()
    beta = np.asarray(inputs["bn_beta_0"], np.float32).reshape(128, 1).copy()

    xf16 = x.astype(f16)
    xlo, xhi = xf16[:HALF].copy(), xf16[HALF:].copy()

    in_maps = []
    for c in range(NCORES):
        xlT = np.zeros((128, NPAD), f16)
        xlT[:, :NLOC] = x[c * NLOC:(c + 1) * NLOC].T.astype(f16)
        im = {
            "xlo": xlo, "xhi": xhi, "xlT": xlT,
            "wself0": wself0, "wn0s": wn0s, "wn0a": wn0a,
            "wself1": wself1, "wn1sa": wn1sa,
            "bias1": bias1, "gamma": gamma, "beta": beta,
        }
        for t in ("s", "a"):
            arr = core_arrays[t][c]
            im[f"idx_{t}_lo"] = arr["idx0"]
            im[f"idx_{t}_hi"] = arr["idx1"]
            im[f"off_{t}"] = arr["off"]
            im[f"w_{t}"] = arr["w"]
        in_maps.append(im)
    return struct, in_maps


def _build(struct):
    import concourse.bacc as bacc
    import concourse.mybir as mybir
    import concourse.tile as tile
    from concourse import bass

    f16, f32 = mybir.dt.float16, mybir.dt.float32
    u16 = mybir.dt.uint16
    nc = bacc.Bacc(None, num_devices=NCORES, dynamic_dma_scratch_size=_DDS)

    din = {}
    def inp(name, shape, dtype):
        din[name] = nc.dram_tensor(name, shape, dtype, kind="ExternalInput")
        return din[name]

    inp("xlo", [HALF, 128], f16)
    inp("xhi", [HALF, 128], f16)
    inp("xlT", [128, NPAD], f16)
    inp("wself0", [128, 128], f16)
    inp("wn0s", [128, 128], f16)
    inp("wn0a", [128, 128], f16)
    inp("wself1", [128, C], f16)
    inp("wn1sa", [128, 2 * C], f16)
    inp("bias1", [128, C], f32)
    inp("gamma", [128, 1], f32)
    inp("beta", [128, 1], f32)
    for t in ("s", "a"):
        st = struct[t]
        inp(f"idx_{t}_lo", [128, max(st["Tlo"], 1) * 8], mybir.dt.int16)
        inp(f"idx_{t}_hi", [128, max(st["Thi"], 1) * 8], mybir.dt.int16)
        inp(f"off_{t}", [128, st["T"]], f32)
        inp(f"w_{t}", [128, st["T"]], f32)
    out_d = nc.dram_tensor("out", [NPAD, C], f32, kind="ExternalOutput")

    with tile.TileContext(nc) as tc, ExitStack() as ctx:
        per = ctx.enter_context(tc.tile_pool(name="per", bufs=1))
        gp = ctx.enter_context(tc.tile_pool(name="gp", bufs=_GBUFS))
        ohp = ctx.enter_context(tc.tile_pool(name="ohp", bufs=_OHBUFS))
        sm = ctx.enter_context(tc.tile_pool(name="sm", bufs=2))
        ps = ctx.enter_context(tc.tile_pool(name="ps", bufs=2, space="PSUM"))
        p1 = ctx.enter_context(tc.tile_pool(name="p1", bufs=2, space="PSUM"))
        dr = ctx.enter_context(tc.tile_pool(name="dr", bufs=1, space="DRAM"))

        def load(name):
            d = din[name]
            t = per.tile(list(d.shape), d.dtype, tag=name)
            nc.sync.dma_start(out=t[:], in_=d[:, :])
            return t

        sb = {k: load(k) for k in
              ["xlT", "wself0", "wn0s", "wn0a", "wself1", "wn1sa",
               "bias1", "gamma", "beta",
               "idx_s_lo", "idx_s_hi", "idx_a_lo", "idx_a_hi",
               "off_s", "w_s", "off_a", "w_a"]}
        iota = per.tile([128, 128], f16, tag="iota")
        ioti = per.tile([128, 128], mybir.dt.int16, tag="ioti")
        nc.gpsimd.iota(ioti[:], pattern=[[1, 128]], base=0,
                       channel_multiplier=0)
        nc.vector.tensor_copy(out=iota[:], in_=ioti[:])

        nireg_cache = {}
        def nireg(v):
            if v not in nireg_cache:
                nireg_cache[v] = nc.gpsimd.to_reg(v)
            return nireg_cache[v]

        hT = per.tile([128, NPAD], f16, tag="hT")
        mean_s = per.tile([128, NPAD], f16, tag="mean_s")
        mean_a = per.tile([128, NPAD], f16, tag="mean_a")
        sums = per.tile([128, NB], f32, tag="sums")
        sumsq = per.tile([128, NB], f32, tag="sumsq")
        stag = per.tile([128, NB * C], f32, tag="stag")
        pstag = per.tile([128, NB * 2 * C], f16, tag="pstag")

        ploc = dr.tile([NPAD, 2 * C], f16)
        pfullc = dr.tile([N, 2 * C], f16)
        pfull = dr.tile([N, 128], f16)
        bnin = dr.tile([128, 2], f32)
        bnout = dr.tile([128, 2], f32)

        def gather_stream(t, tlo_ap, thi_ap):
            """Generator-side gather machinery shared by both layers."""
            st = struct[t]
            idx = {0: sb[f"idx_{t}_lo"], 1: sb[f"idx_{t}_hi"]}
            tot = {0: st["Tlo"], 1: st["Thi"]}
            tabs = {0: tlo_ap, 1: thi_ap}
            gbuf = {0: [], 1: []}
            emitted = {0: 0, 1: 0}

            def ensure(h, batch):
                while emitted[h] <= batch:
                    k = emitted[h]
                    nb_t = min(GB, tot[h] - k * GB)
                    g = gp.tile([128, GB, 128], u16, tag=f"g{h}")
                    nc.gpsimd.dma_gather(
                        out_ap=g[:, :nb_t, :],
                        in_ap=tabs[h],
                        idxs_ap=idx[h][:, k * GB * 8:(k * GB + nb_t) * 8],
                        num_idxs=nb_t * 128,
                        num_idxs_reg=nireg(nb_t * 128),
                        elem_size=128,
                    )
                    gbuf[h].append(g)
                    emitted[h] += 1

            def get(h, j):
                batch, slot = j // GB, j % GB
                ensure(h, batch)
                return gbuf[h][batch], slot

            return get

        def onehot(t, col):
            oh = ohp.tile([128, BLK], f16, tag="oh")
            nc.vector.tensor_scalar(
                out=oh[:], in0=iota[:],
                scalar1=sb[f"off_{t}"][:, col:col + 1],
                scalar2=sb[f"w_{t}"][:, col:col + 1],
                op0=mybir.AluOpType.is_equal,
                op1=mybir.AluOpType.mult)
            return oh

        # ---------------- layer 0: neighbor means + hT + BN stats ------
        # type s fills mean_s; type a loop fuses the per-block hT matmuls
        # and BN stats so the drain overlaps the gather stream.
        for t, mean in (("s", mean_s), ("a", mean_a)):
            get = gather_stream(t, din["xlo"][:, :].bitcast(u16),
                                din["xhi"][:, :].bitcast(u16))
            for b in range(NB):
                ents = struct[t]["entries"][b]
                psum = ps.tile([128, BLK], f32, tag="seg", bufs=_SEGBUFS)
                for k, (h, j, col) in enumerate(ents):
                    g, slot = get(h, j)
                    oh = onehot(t, col)
                    nc.tensor.matmul(out=psum[:],
                                     lhsT=g[:, slot, :].bitcast(f16),
                                     rhs=oh[:], start=(k == 0),
                                     stop=(k == len(ents) - 1))
                cols = slice(b * BLK, (b + 1) * BLK)
                nc.scalar.activation(mean[:, cols], psum[:],
                                     mybir.ActivationFunctionType.Copy)
                if t == "a":
                    po = ps.tile([128, BLK], f32, tag="oT")
                    nc.tensor.matmul(out=po[:], lhsT=sb["wself0"][:],
                                     rhs=sb["xlT"][:, cols],
                                     start=True, stop=False)
                    nc.tensor.matmul(out=po[:], lhsT=sb["wn0s"][:],
                                     rhs=mean_s[:, cols],
                                     start=False, stop=False)
                    nc.tensor.matmul(out=po[:], lhsT=sb["wn0a"][:],
                                     rhs=mean_a[:, cols],
                                     start=False, stop=True)
                    nc.scalar.activation(hT[:, cols], po[:],
                                         mybir.ActivationFunctionType.Copy)
                    nc.vector.tensor_reduce(out=sums[:, b:b + 1],
                                            in_=hT[:, cols],
                                            axis=mybir.AxisListType.X,
                                            op=mybir.AluOpType.add)
                    sq = sm.tile([128, BLK], f32, tag="sq")
                    nc.vector.tensor_tensor(out=sq[:], in0=hT[:, cols],
                                            in1=hT[:, cols],
                                            op=mybir.AluOpType.mult)
                    nc.vector.tensor_reduce(out=sumsq[:, b:b + 1], in_=sq[:],
                                            axis=mybir.AxisListType.X,
                                            op=mybir.AluOpType.add)

        # ---------------- batchnorm ----------------
        bnv = per.tile([128, 2], f32, tag="bnv")
        nc.vector.tensor_reduce(out=bnv[:, 0:1], in_=sums[:],
                                axis=mybir.AxisListType.X,
                                op=mybir.AluOpType.add)
        nc.vector.tensor_reduce(out=bnv[:, 1:2], in_=sumsq[:],
                                axis=mybir.AxisListType.X,
                                op=mybir.AluOpType.add)
        nc.sync.dma_start(out=bnin[:], in_=bnv[:])
        nc.gpsimd.collective_compute(
            "AllReduce", mybir.AluOpType.add,
            replica_groups=[list(range(NCORES))],
            ins=[bnin[:].opt()], outs=[bnout[:].opt()])
        bng = per.tile([128, 2], f32, tag="bng")
        nc.sync.dma_start(out=bng[:], in_=bnout[:])
        mu = per.tile([128, 1], f32, tag="mu")
        ex2 = per.tile([128, 1], f32, tag="ex2")
        var = per.tile([128, 1], f32, tag="var")
        sd = per.tile([128, 1], f32, tag="sd")
        rs = per.tile([128, 1], f32, tag="rs")
        av = per.tile([128, 1], f32, tag="av")
        bv = per.tile([128, 1], f32, tag="bv")
        tmp = per.tile([128, 1], f32, tag="tmp")
        nc.vector.tensor_scalar_mul(mu[:], bng[:, 0:1], 1.0 / N)
        nc.vector.tensor_scalar_mul(ex2[:], bng[:, 1:2], 1.0 / N)
        nc.vector.tensor_tensor(out=tmp[:], in0=mu[:], in1=mu[:],
                                op=mybir.AluOpType.mult)
        nc.vector.tensor_tensor(out=var[:], in0=ex2[:], in1=tmp[:],
                                op=mybir.AluOpType.subtract)
        nc.vector.tensor_scalar_add(var[:], var[:], EPS)
        nc.scalar.activation(sd[:], var[:], mybir.ActivationFunctionType.Sqrt)
        nc.vector.reciprocal(rs[:], sd[:])
        nc.vector.tensor_tensor(out=av[:], in0=sb["gamma"][:], in1=rs[:],
                                op=mybir.AluOpType.mult)
        nc.vector.tensor_tensor(out=tmp[:], in0=av[:], in1=mu[:],
                                op=mybir.AluOpType.mult)
        nc.vector.tensor_tensor(out=bv[:], in0=sb["beta"][:], in1=tmp[:],
                                op=mybir.AluOpType.subtract)
        nc.scalar.activation(hT[:], hT[:], mybir.ActivationFunctionType.Relu,
                             scale=av[:], bias=bv[:])

        # ---------------- projection p = relu(bn(h)) @ [Wn1s|Wn1a] ------
        for b in range(NB):
            cols = slice(b * BLK, (b + 1) * BLK)
            pp = ps.tile([128, 2 * C], f32, tag="pp")
            nc.tensor.matmul(out=pp[:], lhsT=hT[:, cols], rhs=sb["wn1sa"][:],
                             start=True, stop=True)
            nc.scalar.activation(pstag[:, b * 2 * C:(b + 1) * 2 * C], pp[:],
                                 mybir.ActivationFunctionType.Copy)
        nc.sync.dma_start(
            out=ploc[:, :].rearrange("(b p) c -> p b c", p=128),
            in_=pstag[:, :])

        nc.gpsimd.collective_compute(
            "AllGather", mybir.AluOpType.bypass,
            replica_groups=[list(range(NCORES))],
            ins=[ploc[0:NLOC, :].opt()], outs=[pfullc[:].opt()])
        nc.sync.dma_start(out=pfull[:, 0:2 * C], in_=pfullc[:, :])

        # ---------------- layer 1 ----------------
        # sim pass: psum = self + sim entries; stag = psum + bias
        # anc pass: psum = anc entries;        stag = stag + psum
        for t in ("s", "a"):
            get = gather_stream(
                t,
                pfull[0:HALF, :].bitcast(u16),
                pfull[HALF:N, :].bitcast(u16))
            fsl = slice(0, C) if t == "s" else slice(C, 2 * C)
            for b in range(NB):
                cols = slice(b * BLK, (b + 1) * BLK)
                osl = slice(b * C, (b + 1) * C)
                ents = struct[t]["entries"][b]
                pf = p1.tile([128, C], f32, tag="pf")
                if t == "s":
                    nc.tensor.matmul(out=pf[:], lhsT=hT[:, cols],
                                     rhs=sb["wself1"][:],
                                     start=True, stop=(len(ents) == 0))
                for k, (h, j, col) in enumerate(ents):
                    g, slot = get(h, j)
                    oh = onehot(t, col)
                    nc.tensor.matmul(out=pf[:], lhsT=oh[:],
                                     rhs=g[:, slot, fsl].bitcast(f16),
                                     start=(t == "a" and k == 0),
                                     stop=(k == len(ents) - 1))
                if t == "s":
                    nc.vector.tensor_tensor(out=stag[:, osl], in0=pf[:],
                                            in1=sb["bias1"][:],
                                            op=mybir.AluOpType.add)
                else:
                    nc.vector.tensor_tensor(out=stag[:, osl], in0=stag[:, osl],
                                            in1=pf[:],
                                            op=mybir.AluOpType.add)
        nc.sync.dma_start(
            out=out_d[:, :].rearrange("(b p) c -> p b c", p=128),
            in_=stag[:, :])
    nc.compile()
    return nc


def kernel(**inputs):
    from concourse.bass_utils import run_bass_kernel_spmd

    struct, in_maps = _prepare(inputs)
    key = (tuple(sorted(struct["s"]["nslots"].items())),
           tuple(sorted(struct["a"]["nslots"].items())))
    if key not in _CACHE:
        _CACHE.clear()
        _CACHE[key] = _build(struct)
    nc = _CACHE[key]
    res = run_bass_kernel_spmd(nc, in_maps, core_ids=list(range(NCORES)))
    return np.concatenate([res.results[c]["out"][:NLOC] for c in range(NCORES)],
                          axis=0)
